# revision 19
# baseline (speedup 1.0000x reference)
"""DNC addressing kernel for Trainium2, 8 NeuronCores, batch-sharded.

Math reformulations vs the reference (numerically validated):
  * directional: the [B,N,N] shift kernel is circulant with row-constant
    normalization; dw[m] = sum_j gn[j] * w[(m-1024+j) % N] with j <= 15
    (Gaussian taps decay below f32 eps past j=6 even at max |sc|).
  * allocation: alloc[p] = exp(G_p + L_p), L = log1p(-u),
    G_p = sum over q with u_q < u_p of L_q (host nudge makes u unique).
    Only the ~210 smallest-usage entries per row give alloc above ~1e-6
    (tolerance is 2e-2), so the smallest entries (u < T) are compacted
    with the gpsimd sparse_gather ucode (exact for u that are multiples
    of 2^-23, which the inputs are), compared against all 2048 thresholds
    with DVE tensor_scalar is_ge masks (fp16 {0,1}), and reduced with
    mask-as-lhsT fp16 PE matmuls giving exp-ready G+L directly in cm
    layout on 128 psum partitions. Thresholds >= T fall out as
    exp(G_total) with error < 3e-4.

Layouts: "rm" means n = p*16 + c (contiguous 64B runs per partition, fast
DMA), "cm" means n = c*128 + p (the mask column order). alloc converts
cm->rm via PE transpose + a DRAM round-trip through its own output.
"""

import sys

for _p in ("/opt/trn_rl_repo", "/root/.axon_site/_ro/trn_rl_repo"):
    if _p not in sys.path:
        sys.path.append(_p)

import numpy as np

import concourse.bass as bass
import concourse.mybir as mybir
from bass_rust import AP
from concourse.tile import TileContext
from concourse import library_config
from concourse.library_overlay import lower_extended_insts

F32 = mybir.dt.float32
F16 = mybir.dt.float16
U32 = mybir.dt.uint32
AF = mybir.ActivationFunctionType
ALU = mybir.AluOpType
AX = mybir.AxisListType

NCORES = 8
B, N, W, C = 32, 2048, 64, 1024
BL = B // NCORES          # 4 rows per core
P = 128                   # partitions
NCH = N // P              # 16 cm chunks
KT = 16                   # directional taps
EPS = 1e-8

UT = 0.09                 # usage compaction cutoff
KCAP = 256                # compacted slot capacity (16x16 tile)
KCH = KCAP // P           # 2 element chunks of 128 slots

_CACHE = {}


def _split_waits(nc, cap=1):
    """Walrus codegen rejects instructions with more than ~1 semaphore wait
    (PE load-weights fails at 2). Hoist excess waits onto same-engine NOPs
    inserted just before the instruction."""
    import bass_rust

    wid = [0]
    for f in nc.m.functions:
        for blk in f.blocks:
            new = []
            for inst in blk.instructions:
                si = inst.sync_info
                waits = list(si.on_wait) if si is not None and si.on_wait else []
                if len(waits) > cap:
                    keep = waits[-cap:]
                    extra = waits[:-cap]
                    for i in range(0, len(extra), cap):
                        nop = bass_rust.InstNoOp(
                            name=f"WNOP-{wid[0]}", ins=[], outs=[])
                        wid[0] += 1
                        nop.engine = inst.engine
                        nop.sync_info = mybir.SyncInfo(
                            on_wait=extra[i:i + cap], on_update=[])
                        new.append(nop)
                    inst.sync_info = mybir.SyncInfo(
                        on_wait=keep, on_update=si.on_update)
                new.append(inst)
            blk.instructions[:] = new


def _win(ap, dims):
    """Raw windowed view of an SBUF tile AP: keep partition dim, replace the
    free dims (overlapping windows allowed)."""
    return AP(tensor=ap.tensor, offset=ap.offset, ap=[ap.ap[0]] + dims)


def _build():
    nc = bass.Bass()

    mem_d = nc.dram_tensor("mem", [BL, N, W], F16, kind="ExternalInput")
    coT_d = nc.dram_tensor("coT", [C, BL], F32, kind="ExternalInput")
    wcat_d = nc.dram_tensor("wcat", [C, 69], F32, kind="ExternalInput")
    bcat_d = nc.dram_tensor("bcat", [BL, 69], F32, kind="ExternalInput")
    wext_d = nc.dram_tensor("wext", [BL, N + KT - 1], F16, kind="ExternalInput")
    u_d = nc.dram_tensor("u", [BL, N], F32, kind="ExternalInput")
    ksqn_d = nc.dram_tensor("ksqn", [BL, KT], F32, kind="ExternalInput")
    ident_d = nc.dram_tensor("ident", [P, P], F32, kind="ExternalInput")
    slotid_d = nc.dram_tensor("slotid", [P, KCH], F32, kind="ExternalInput")

    o_ww = nc.dram_tensor("o_ww", [BL, N], F32, kind="ExternalOutput")
    o_cw = nc.dram_tensor("o_cw", [BL, N], F32, kind="ExternalOutput")
    o_dw = nc.dram_tensor("o_dw", [BL, N], F32, kind="ExternalOutput")
    o_al = nc.dram_tensor("o_al", [BL, N], F32, kind="ExternalOutput")

    kb_s = nc.dram_tensor("kb_s", [BL * W], F32, kind="Internal")
    gn_s = nc.dram_tensor("gn_s", [BL * KT], F32, kind="Internal")
    wh_s = nc.dram_tensor("wh_s", [BL], F32, kind="Internal")
    ucs_s = nc.dram_tensor("ucs_s", [BL * KCAP], F32, kind="Internal")

    with TileContext(nc) as tc:
        with tc.tile_pool(name="sb", bufs=1) as pool, \
             tc.tile_pool(name="ps", bufs=1, space="PSUM") as ppool:

            dma = nc.sync.dma_start      # HWDGE engine 1
            dma2 = nc.scalar.dma_start   # HWDGE engine 2

            nc.gpsimd.load_library(library_config.sparse_gather)

            # ---------------- phase E head: compaction of small usage -----
            # u16 wrap order is arbitrary (values only), so use the
            # DMA-friendly p-major mapping; one fused load for all rows.
            # Everything the gathers need is emitted FIRST on each engine.
            u16a = pool.tile([16, BL, P], F32, tag="u16a")
            dma2(out=u16a[:], in_=AP(tensor=u_d, offset=0,
                                     ap=[[P, 16], [N, BL], [1, P]]))

            # um = u - 2*(u >= T): keeps u<T, maps the rest negative
            m2a = pool.tile([16, BL, P], F32, tag="m2a")
            nc.vector.tensor_scalar(out=m2a[:], in0=u16a[:], scalar1=UT,
                                    scalar2=-2.0, op0=ALU.is_ge, op1=ALU.mult)
            uma = pool.tile([16, BL, P], F32, tag="uma")
            nc.vector.tensor_add(uma[:], m2a[:], u16a[:])
            ucpts = []
            for r in range(BL):
                ucpt = pool.tile([16, KCAP // 16], F32, tag=f"ucpt_{r}")
                nc.vector.memset(ucpt[:], 0.5)  # hw ucode rewrites the tail
                ucpts.append(ucpt)

            nfs = pool.tile([1, BL], U32, tag="nfs")
            for r in range(BL):
                nc.gpsimd.sparse_gather(ucpts[r][:], uma[:, r, :],
                                        num_found=nfs[:, r:r + 1])

            # ---- input triggers: sync gets the early/critical, scalar the rest
            coT_ld = pool.tile([P, C // P, BL], F32, tag="coT_ld")
            dma2(out=coT_ld[:], in_=AP(tensor=coT_d, offset=0,
                                       ap=[[BL, P], [P * BL, C // P], [1, BL]]))
            wcat_ld = pool.tile([P, C // P, 69], F32, tag="wcat_ld")
            dma2(out=wcat_ld[:], in_=AP(tensor=wcat_d, offset=0,
                                        ap=[[69, P], [P * 69, C // P],
                                            [1, 69]]))
            u_b_all = pool.tile([P, BL * N], F32, tag="u_b_all")
            for r in range(BL):
                dma(out=u_b_all[:, r * N:(r + 1) * N],
                    in_=AP(tensor=u_d, offset=r * N, ap=[[0, P], [1, N]]))
            memts = []
            for h in range(2):
                memt2 = pool.tile([P, 2, NCH, W], F16, tag=f"memt_{h}")
                memts.append(memt2)
            dma(out=memts[0][:],
                in_=AP(tensor=mem_d, offset=0,
                       ap=[[NCH * W, P], [N * W, 2], [W, NCH], [1, W]]))

            bcat_sb = pool.tile([BL, 69], F32, tag="bcat")
            dma2(out=bcat_sb[:], in_=bcat_d[:])
            ksqn_sb = pool.tile([BL, KT], F32, tag="ksqn")
            dma2(out=ksqn_sb[:], in_=ksqn_d[:])
            ident_sb = pool.tile([P, P], F32, tag="ident")
            dma2(out=ident_sb[:], in_=ident_d[:])
            slotid_sb = pool.tile([P, KCH], F32, tag="slotid")
            dma2(out=slotid_sb[:], in_=slotid_d[:])
            dma(out=memts[1][:],
                in_=AP(tensor=mem_d, offset=2 * N * W,
                       ap=[[NCH * W, P], [N * W, 2], [W, NCH], [1, W]]))
            vsba = pool.tile([P, BL, NCH + KT - 1], F16, tag="vsba")
            dma(out=vsba[:], in_=AP(tensor=wext_d, offset=0,
                                     ap=[[NCH, P], [N + KT - 1, BL],
                                         [1, NCH + KT - 1]]))

            # ---- DVE consts + fp16 bounces for the phase-A matmuls
            ones1 = pool.tile([1, P], F32, tag="ones1")
            nc.vector.memset(ones1[:], 1.0)
            ones16 = pool.tile([P, 1], F16, tag="ones16")
            nc.vector.memset(ones16[:], 1.0)
            ones_sb = pool.tile([P, 1], F32, tag="ones")
            nc.vector.memset(ones_sb[:], 1.0)
            eps_t = pool.tile([BL, 1], F32, tag="eps")
            nc.vector.memset(eps_t[:], float(EPS))
            coT_sb = pool.tile([P, C // P, BL], F16, tag="coT")
            nc.vector.tensor_copy(coT_sb[:], coT_ld[:])
            wcat_sb = pool.tile([P, C // P, 69], F16, tag="wcat")
            nc.vector.tensor_copy(wcat_sb[:], wcat_ld[:])

            # ---- PE: phase A matmuls first (idle otherwise)
            psA = ppool.tile([BL, 69], F32, tag="psA")
            for k in range(C // P):
                nc.tensor.matmul(psA[:], coT_sb[:, k, :], wcat_sb[:, k, :],
                                 start=(k == 0), stop=(k == C // P - 1))

            # ---- compaction bounces (scalar queue; park on the gathers)
            ucm2s = []
            for r in range(BL):
                dma2(out=AP(tensor=ucs_s, offset=r * KCAP,
                            ap=[[16, 16], [1, 16]]), in_=ucpts[r][:])
                ucm2 = pool.tile([P, KCH], F32, tag=f"ucm2_{r}")
                dma2(out=ucm2[:], in_=AP(tensor=ucs_s, offset=r * KCAP,
                                         ap=[[KCH, P], [1, KCH]]))
                ucm2s.append(ucm2)

            # ---- phase A head (deps land ~12us; scalar loads its tables)
            zs = pool.tile([BL, 69], F32, tag="zs")
            nc.vector.tensor_add(zs[:], psA[:], bcat_sb[:])
            z3 = zs[:, W + 1:W + 4]
            z3m = pool.tile([BL, 1], F32, tag="z3m")
            nc.vector.reduce_max(z3m[:], z3, axis=AX.X)
            kt_t = pool.tile([BL, W], F32, tag="kt")
            nc.scalar.activation(kt_t[:], zs[:, 0:W], AF.Tanh)
            # softplus via exp + ln(1+x): no Softplus act-table in this build
            bexp = pool.tile([BL, 1], F32, tag="bexp")
            nc.scalar.activation(bexp[:], zs[:, W:W + 1], AF.Exp)
            beta = pool.tile([BL, 1], F32, tag="beta")
            nc.scalar.activation(beta[:], bexp[:], AF.Ln, bias=1.0)

            # ---------------- phase E tail: masks + PE reduce per row ------
            # Phase-A scraps are interleaved between rows so neither DVE nor
            # the scalar engine ever parks in front of the row pipeline.
            pms, psGs = [], []
            for r in range(BL):
                ucm2 = ucm2s[r]
                # Ltil = ln(clamp(1-u, tiny, 1)); garbage slots contribute 0
                omu = pool.tile([P, KCH], F32, tag=f"omu_{r}")
                nc.vector.tensor_scalar(out=omu[:], in0=ucm2[:], scalar1=1.0,
                                        scalar2=-1.0, op0=ALU.subtract,
                                        op1=ALU.mult)
                omc = pool.tile([P, KCH], F32, tag=f"omc_{r}")
                nc.vector.tensor_scalar(out=omc[:], in0=omu[:], scalar1=1e-30,
                                        scalar2=1.0, op0=ALU.max, op1=ALU.min)
                L32 = pool.tile([P, KCH], F32, tag=f"L32_{r}")
                nc.scalar.activation(L32[:], omc[:], AF.Ln)
                # hw ucode rewrites the whole output tile from its internal
                # scratch: tail slots hold the PREVIOUS call's compacted
                # values. Zero L beyond num_found so they contribute nothing.
                nf_f = pool.tile([1, 1], F32, tag=f"nf_f{r}")
                nc.vector.tensor_copy(nf_f[:], nfs[:, r:r + 1])
                psNf = ppool.tile([P, 1], F32, tag="psNf")
                nc.tensor.matmul(psNf[:], ones1[:], nf_f[:], start=True,
                                 stop=True)
                valid = pool.tile([P, KCH], F32, tag=f"valid_{r}")
                nc.vector.tensor_scalar(out=valid[:], in0=slotid_sb[:],
                                        scalar1=psNf[:], scalar2=None,
                                        op0=ALU.is_lt)
                L32c = pool.tile([P, KCH], F32, tag=f"L32c_{r}")
                nc.vector.tensor_mul(L32c[:], L32[:], valid[:])

                # maskedL: L_slot * [t_p >= u_slot] (is_ge includes the self
                # L term); pair-sum the two slot chunks so PE sees one lhsT
                ml0 = pool.tile([P, N], F16, tag="ml0")
                nc.vector.tensor_scalar(
                    out=ml0[:], in0=u_b_all[:, r * N:(r + 1) * N],
                    scalar1=ucm2[:, 0:1], scalar2=L32c[:, 0:1],
                    op0=ALU.is_ge, op1=ALU.mult)
                ml1 = pool.tile([P, N], F16, tag="ml1")
                nc.vector.tensor_scalar(
                    out=ml1[:], in0=u_b_all[:, r * N:(r + 1) * N],
                    scalar1=ucm2[:, 1:2], scalar2=L32c[:, 1:2],
                    op0=ALU.is_ge, op1=ALU.mult)
                pm = pool.tile([P, N], F16, tag=f"pm{r % 2}")
                nc.vector.tensor_add(pm[:], ml0[:], ml1[:])
                pms.append(pm)

                # G+L in cm layout on psum partitions via maskedL-as-lhsT
                psG = ppool.tile([P, NCH], F32, tag=f"psG{r % 2}")
                for t in range(NCH):
                    nc.tensor.matmul(psG[:, t:t + 1], pm[:, t * P:(t + 1) * P],
                                     ones16[:], start=True, stop=True)
                psGs.append(psG)

                # phase-A scraps fill the gaps between rows
                if r == 0:
                    kb = pool.tile([BL, W], F32, tag="kb")
                    nc.vector.tensor_scalar_mul(kb[:], kt_t[:], beta[:])
                    nz3 = pool.tile([BL, 1], F32, tag="nz3")
                    nc.scalar.mul(nz3[:], z3m[:], -1.0)
                    e3 = pool.tile([BL, 3], F32, tag="e3")
                    nc.scalar.activation(e3[:], z3, AF.Exp, bias=nz3[:])
                elif r == 1:
                    s3 = pool.tile([BL, 1], F32, tag="s3")
                    nc.vector.reduce_sum(s3[:], e3[:], axis=AX.X)
                    r3 = pool.tile([BL, 1], F32, tag="r3")
                    nc.vector.reciprocal(r3[:], s3[:])
                    scr = pool.tile([BL, 1], F32, tag="scr")
                    nc.vector.tensor_sub(scr[:], e3[:, 2:3], e3[:, 0:1])
                    sc = pool.tile([BL, 1], F32, tag="sc")
                    nc.vector.tensor_mul(sc[:], scr[:], r3[:])
                    sq = pool.tile([BL, 1], F32, tag="sq")
                    nc.scalar.square(sq[:], sc[:])
                    tau = pool.tile([BL, 1], F32, tag="tau")
                    nc.scalar.activation(tau[:], sq[:], AF.Identity,
                                         bias=eps_t[:], scale=2.0)
                    wgt = pool.tile([BL, 1], F32, tag="wgt")
                    nc.scalar.activation(wgt[:], zs[:, W + 4:W + 5],
                                         AF.Sigmoid)
                    wh = pool.tile([BL, 1], F32, tag="wh")
                    nc.scalar.mul(wh[:], wgt[:], 0.5)
                    dma2(out=kb_s[:].rearrange("(r w) -> r w", r=BL),
                         in_=kb[:])
                    dma2(out=wh_s[:].rearrange("(r o) -> r o", r=BL),
                         in_=wh[:])
                elif r == 2:
                    rtau = pool.tile([BL, 1], F32, tag="rtau")
                    nc.vector.reciprocal(rtau[:], tau[:])
                    garg = pool.tile([BL, KT], F32, tag="garg")
                    nc.vector.tensor_scalar_mul(garg[:], ksqn_sb[:], rtau[:])
                    g_t = pool.tile([BL, KT], F32, tag="g")
                    nc.scalar.activation(g_t[:], garg[:], AF.Exp)
                else:
                    S_t = pool.tile([BL, 1], F32, tag="S")
                    nc.vector.reduce_sum(S_t[:], g_t[:], axis=AX.X)
                    Se = pool.tile([BL, 1], F32, tag="Se")
                    nc.scalar.activation(Se[:], S_t[:], AF.Identity,
                                         bias=eps_t[:])
                    rS = pool.tile([BL, 1], F32, tag="rS")
                    nc.vector.reciprocal(rS[:], Se[:])
                    gn = pool.tile([BL, KT], F32, tag="gn")
                    nc.vector.tensor_scalar_mul(gn[:], g_t[:], rS[:])
                    dma2(out=gn_s[:].rearrange("(r j) -> r j", r=BL),
                         in_=gn[:])
                    kb_ba = pool.tile([P, BL, W], F32, tag="kb_ba")
                    dma2(out=kb_ba[:], in_=AP(tensor=kb_s, offset=0,
                                              ap=[[0, P], [1, BL * W]]))
                    gnb = pool.tile([P, BL, KT], F32, tag="gnb")
                    dma2(out=gnb[:], in_=AP(tensor=gn_s, offset=0,
                                            ap=[[0, P], [KT, BL], [1, KT]]))
                    whb = pool.tile([P, BL], F32, tag="whb")
                    dma2(out=whb[:], in_=AP(tensor=wh_s, offset=0,
                                            ap=[[0, P], [1, BL]]))

            # ---- per-row epilogue: exp, transpose, rm bounce
            al_rms = []
            for r in range(BL):
                al_cm = pool.tile([P, NCH], F32, tag=f"alcm_{r % 2}")
                nc.scalar.activation(al_cm[:], psGs[r][:], AF.Exp)
                # cm -> rm via PE transpose + DRAM round-trip
                psT = ppool.tile([NCH, P], F32, tag=f"psT{r % 2}")
                nc.tensor.transpose(psT[:], al_cm[:], ident_sb[:])
                alT = pool.tile([NCH, P], F32, tag=f"alT_{r % 2}")
                nc.scalar.copy(alT[:], psT[:])
                dma(out=AP(tensor=o_al, offset=r * N,
                           ap=[[P, NCH], [1, P]]), in_=alT[:])
                al_rm = pool.tile([P, NCH], F32, tag=f"alrm{r}")
                dma(out=al_rm[:], in_=AP(tensor=o_al, offset=r * N,
                                         ap=[[NCH, P], [1, NCH]]))
                al_rms.append(al_rm)

            # ---------------- phase B: sim = mem . (k*beta), fp16, rm ------
            sim_all = pool.tile([P, BL, NCH], F32, tag="sim_all")
            kb16a = pool.tile([P, BL, W], F16, tag="kb16a")
            nc.vector.tensor_copy(kb16a[:], kb_ba[:])
            for r in range(BL):
                smul = pool.tile([P, NCH, W], F16, tag=f"smul{r % 2}")
                nc.vector.tensor_mul(
                    smul[:], memts[r // 2][:, r % 2, :, :],
                    kb16a[:, r, :].unsqueeze(1).broadcast_to([P, NCH, W]))
                nc.vector.tensor_reduce(sim_all[:, r, :], smul[:], axis=AX.X,
                                        op=ALU.add)

            # ---------------- phase C: content softmax (no max-shift) -----
            e_cm = pool.tile([P, BL, NCH], F32, tag="e_cm")
            nc.scalar.activation(e_cm[:], sim_all[:], AF.Exp)
            esum = pool.tile([P, BL], F32, tag="esum")
            nc.vector.tensor_reduce(esum[:], e_cm[:], axis=AX.X, op=ALU.add)
            psC = ppool.tile([1, BL], F32, tag="psC")
            nc.tensor.matmul(psC[:], ones_sb[:], esum[:], start=True, stop=True)
            rCs = pool.tile([1, BL], F32, tag="rCs")
            nc.vector.reciprocal(rCs[:], psC[:])
            rsb = ppool.tile([P, BL], F32, tag="rsb")
            nc.tensor.matmul(rsb[:], ones1[:], rCs[:], start=True, stop=True)

            # ---------------- phase D: directional (16-tap), fp16, rm ------
            dw_all = pool.tile([P, BL, NCH], F32, tag="dw_all")
            gnb16 = pool.tile([P, BL, KT], F16, tag="gnb16")
            nc.vector.tensor_copy(gnb16[:], gnb[:])
            for r in range(BL):
                dmul = pool.tile([P, NCH, KT], F16, tag=f"dmul{r}")
                nc.vector.tensor_mul(
                    dmul[:], _win(vsba[:, r, :], [[1, NCH], [1, KT]]),
                    gnb16[:, r:r + 1, :].broadcast_to([P, NCH, KT]))
                nc.vector.tensor_reduce(dw_all[:, r, :], dmul[:], axis=AX.X,
                                        op=ALU.add)

            # ---------------- phase F: combine + fused stores (rm) ---------
            rm_all = lambda d: AP(tensor=d, offset=0,
                                  ap=[[NCH, P], [N, BL], [1, NCH]])
            dma2(out=rm_all(o_dw), in_=dw_all[:])
            cw_all = pool.tile([P, BL, NCH], F32, tag="cw_all")
            ww_all = pool.tile([P, BL, NCH], F32, tag="ww_all")
            for r in range(BL):
                nc.vector.tensor_scalar_mul(cw_all[:, r, :], e_cm[:, r, :],
                                            rsb[:, r:r + 1])
                dwal = pool.tile([P, NCH], F32, tag=f"dwal{r}")
                nc.vector.tensor_mul(dwal[:], dw_all[:, r, :], al_rms[r][:])
                tsum = pool.tile([P, NCH], F32, tag=f"tsum{r}")
                nc.vector.tensor_add(tsum[:], cw_all[:, r, :], dwal[:])
                nc.vector.tensor_scalar_mul(ww_all[:, r, :], tsum[:],
                                            whb[:, r:r + 1])
            dma(out=rm_all(o_cw), in_=cw_all[:])
            dma2(out=rm_all(o_ww), in_=ww_all[:])

    _split_waits(nc)
    lower_extended_insts(nc)
    return nc


def _dedup_rows(u):
    """Make every row's values unique by bumping later duplicates up in
    2^-23 quanta (the sparse_gather ucode's fixed-point grid). Matches the
    reference's argsort lex order to ~1e-7."""
    q = np.float32(2.0 ** -23)
    u = u.copy()
    for r in range(u.shape[0]):
        row = u[r]
        for _ in range(8):
            vals, counts = np.unique(row, return_counts=True)
            dups = vals[counts > 1]
            if dups.size == 0:
                break
            for v in dups:
                idx = np.flatnonzero(row == v)[1:]
                for j, p in enumerate(idx):
                    row[p] = v + np.float32(j + 1) * q
    return u


def _host_prep(inputs):
    co = np.ascontiguousarray(inputs["controller_output"], dtype=np.float32)
    prw = np.ascontiguousarray(inputs["prev_read_weights"], dtype=np.float32)
    memory = np.ascontiguousarray(inputs["memory"], dtype=np.float32)
    usage = _dedup_rows(np.asarray(inputs["usage"], dtype=np.float32))

    cnt = (usage < UT).sum(axis=1)
    assert cnt.max() <= KCAP, f"compaction overflow: {cnt.max()} > {KCAP}"

    wcat = np.concatenate([np.asarray(inputs["Wk"]), np.asarray(inputs["Wb"]),
                           np.asarray(inputs["Ws"]), np.asarray(inputs["Wg"])],
                          axis=0).T  # [C, 69]
    wcat = np.ascontiguousarray(wcat, dtype=np.float32)
    bcat = np.concatenate([np.asarray(inputs["bk"]), np.asarray(inputs["bb"]),
                           np.asarray(inputs["bs"]),
                           np.asarray(inputs["bg"])]).astype(np.float32)
    bcat_rep = np.ascontiguousarray(np.broadcast_to(bcat, (BL, 69)))

    # v[m] = w[(m-1024) % N]; extended with KT-1 wrap elements
    v = np.concatenate([prw[:, N // 2:], prw[:, :N // 2]], axis=1)
    wext = np.ascontiguousarray(
        np.concatenate([v, v[:, :KT - 1]], axis=1).astype(np.float16))

    ident = np.eye(P, dtype=np.float32)
    ksqn = np.ascontiguousarray(np.broadcast_to(
        -(np.arange(KT, dtype=np.float32) ** 2), (BL, KT)), dtype=np.float32)
    # slot order: gather writes slot i at (partition i%16, free i//16) of the
    # [16, KCAP/16] tile; the DRAM bounce stores linear j = p*16+f and the
    # cm2 reload maps j = 2*p2+k. slotid = gather index of each (p2, k).
    j = (np.arange(P, dtype=np.int64)[:, None] * KCH
         + np.arange(KCH, dtype=np.int64)[None, :])
    slotid = np.ascontiguousarray(
        ((j % 16) * (KCAP // 16) + j // 16).astype(np.float32))

    in_maps = []
    for cidx in range(NCORES):
        rows = slice(cidx * BL, (cidx + 1) * BL)
        in_maps.append({
            "mem": np.ascontiguousarray(memory[rows].astype(np.float16)),
            "coT": np.ascontiguousarray(co[rows].T),
            "wcat": wcat,
            "bcat": bcat_rep,
            "wext": np.ascontiguousarray(wext[rows]),
            "u": np.ascontiguousarray(usage[rows]),
            "ksqn": ksqn,
            "ident": ident,
            "slotid": slotid,
        })
    return in_maps


def kernel(**inputs):
    return _run(inputs, trace=False)[0]


def _run(inputs, trace=False):
    from concourse.bass_utils import run_bass_kernel_spmd

    if "nc" not in _CACHE:
        _CACHE["nc"] = _build()
    nc = _CACHE["nc"]

    in_maps = _host_prep(inputs)
    res = run_bass_kernel_spmd(nc, in_maps, core_ids=list(range(NCORES)),
                               trace=trace)

    ww = np.concatenate([res.results[i]["o_ww"] for i in range(NCORES)], axis=0)
    cw = np.concatenate([res.results[i]["o_cw"] for i in range(NCORES)], axis=0)
    dw = np.concatenate([res.results[i]["o_dw"] for i in range(NCORES)], axis=0)
    al = np.concatenate([res.results[i]["o_al"] for i in range(NCORES)], axis=0)
    out = (ww.astype(np.float32), cw.astype(np.float32),
           dw.astype(np.float32), al.astype(np.float32))
    return out, res


# revision 20
# speedup vs baseline: 1.0226x; 1.0226x over previous
"""DNC addressing kernel for Trainium2, 8 NeuronCores, batch-sharded.

Math reformulations vs the reference (numerically validated):
  * directional: the [B,N,N] shift kernel is circulant with row-constant
    normalization; dw[m] = sum_j gn[j] * w[(m-1024+j) % N] with j <= 15
    (Gaussian taps decay below f32 eps past j=6 even at max |sc|).
  * allocation: alloc[p] = exp(G_p + L_p), L = log1p(-u),
    G_p = sum over q with u_q < u_p of L_q (host nudge makes u unique).
    Only the ~210 smallest-usage entries per row give alloc above ~1e-6
    (tolerance is 2e-2), so the smallest entries (u < T) are compacted
    with the gpsimd sparse_gather ucode (exact for u that are multiples
    of 2^-23, which the inputs are), compared against all 2048 thresholds
    with DVE tensor_scalar is_ge masks (fp16 {0,1}), and reduced with
    mask-as-lhsT fp16 PE matmuls giving exp-ready G+L directly in cm
    layout on 128 psum partitions. Thresholds >= T fall out as
    exp(G_total) with error < 3e-4.

Layouts: "rm" means n = p*16 + c (contiguous 64B runs per partition, fast
DMA), "cm" means n = c*128 + p (the mask column order). alloc converts
cm->rm via PE transpose + a DRAM round-trip through its own output.
"""

import sys

for _p in ("/opt/trn_rl_repo", "/root/.axon_site/_ro/trn_rl_repo"):
    if _p not in sys.path:
        sys.path.append(_p)

import numpy as np

import concourse.bass as bass
import concourse.mybir as mybir
from bass_rust import AP
from concourse.tile import TileContext
from concourse import library_config
from concourse.library_overlay import lower_extended_insts

F32 = mybir.dt.float32
F16 = mybir.dt.float16
U32 = mybir.dt.uint32
AF = mybir.ActivationFunctionType
ALU = mybir.AluOpType
AX = mybir.AxisListType

NCORES = 8
B, N, W, C = 32, 2048, 64, 1024
BL = B // NCORES          # 4 rows per core
P = 128                   # partitions
NCH = N // P              # 16 cm chunks
KT = 16                   # directional taps
EPS = 1e-8

UT = 0.09                 # usage compaction cutoff
KCAP = 256                # compacted slot capacity (16x16 tile)
KCH = KCAP // P           # 2 element chunks of 128 slots

_CACHE = {}


def _split_waits(nc, cap=1):
    """Walrus codegen rejects instructions with more than ~1 semaphore wait
    (PE load-weights fails at 2). Hoist excess waits onto same-engine NOPs
    inserted just before the instruction."""
    import bass_rust

    wid = [0]
    for f in nc.m.functions:
        for blk in f.blocks:
            new = []
            for inst in blk.instructions:
                si = inst.sync_info
                waits = list(si.on_wait) if si is not None and si.on_wait else []
                if len(waits) > cap:
                    keep = waits[-cap:]
                    extra = waits[:-cap]
                    for i in range(0, len(extra), cap):
                        nop = bass_rust.InstNoOp(
                            name=f"WNOP-{wid[0]}", ins=[], outs=[])
                        wid[0] += 1
                        nop.engine = inst.engine
                        nop.sync_info = mybir.SyncInfo(
                            on_wait=extra[i:i + cap], on_update=[])
                        new.append(nop)
                    inst.sync_info = mybir.SyncInfo(
                        on_wait=keep, on_update=si.on_update)
                new.append(inst)
            blk.instructions[:] = new


def _win(ap, dims):
    """Raw windowed view of an SBUF tile AP: keep partition dim, replace the
    free dims (overlapping windows allowed)."""
    return AP(tensor=ap.tensor, offset=ap.offset, ap=[ap.ap[0]] + dims)


def _build():
    nc = bass.Bass()

    mem_d = nc.dram_tensor("mem", [BL, N, W], F16, kind="ExternalInput")
    coT_d = nc.dram_tensor("coT", [C, BL], F32, kind="ExternalInput")
    wcat_d = nc.dram_tensor("wcat", [C, 69], F32, kind="ExternalInput")
    bcat_d = nc.dram_tensor("bcat", [BL, 69], F32, kind="ExternalInput")
    wext_d = nc.dram_tensor("wext", [BL, N + KT - 1], F16, kind="ExternalInput")
    u_d = nc.dram_tensor("u", [BL, N], F32, kind="ExternalInput")
    ksqn_d = nc.dram_tensor("ksqn", [BL, KT], F32, kind="ExternalInput")
    ident_d = nc.dram_tensor("ident", [P, P], F32, kind="ExternalInput")
    slotid_d = nc.dram_tensor("slotid", [P, KCH], F32, kind="ExternalInput")

    o_ww = nc.dram_tensor("o_ww", [BL, N], F32, kind="ExternalOutput")
    o_cw = nc.dram_tensor("o_cw", [BL, N], F32, kind="ExternalOutput")
    o_dw = nc.dram_tensor("o_dw", [BL, N], F32, kind="ExternalOutput")
    o_al = nc.dram_tensor("o_al", [BL, N], F32, kind="ExternalOutput")

    kb_s = nc.dram_tensor("kb_s", [BL * W], F32, kind="Internal")
    gn_s = nc.dram_tensor("gn_s", [BL * KT], F32, kind="Internal")
    wh_s = nc.dram_tensor("wh_s", [BL], F32, kind="Internal")
    ucs_s = nc.dram_tensor("ucs_s", [BL * KCAP], F32, kind="Internal")

    with TileContext(nc) as tc:
        with tc.tile_pool(name="sb", bufs=1) as pool, \
             tc.tile_pool(name="ps", bufs=1, space="PSUM") as ppool:

            dma = nc.sync.dma_start      # HWDGE engine 1
            dma2 = nc.scalar.dma_start   # HWDGE engine 2

            nc.gpsimd.load_library(library_config.sparse_gather)

            # ---------------- phase E head: compaction of small usage -----
            # u16 wrap order is arbitrary (values only), so use the
            # DMA-friendly p-major mapping; one fused load for all rows.
            # Everything the gathers need is emitted FIRST on each engine.
            u16a = pool.tile([16, BL, P], F32, tag="u16a")
            dma2(out=u16a[:], in_=AP(tensor=u_d, offset=0,
                                     ap=[[P, 16], [N, BL], [1, P]]))

            # um = u - 2*(u >= T): keeps u<T, maps the rest negative
            m2a = pool.tile([16, BL, P], F32, tag="m2a")
            nc.vector.tensor_scalar(out=m2a[:], in0=u16a[:], scalar1=UT,
                                    scalar2=-2.0, op0=ALU.is_ge, op1=ALU.mult)
            uma = pool.tile([16, BL, P], F32, tag="uma")
            nc.vector.tensor_add(uma[:], m2a[:], u16a[:])
            ucpts = []
            for r in range(BL):
                ucpt = pool.tile([16, KCAP // 16], F32, tag=f"ucpt_{r}")
                nc.vector.memset(ucpt[:], 0.5)  # hw ucode rewrites the tail
                ucpts.append(ucpt)

            nfs = pool.tile([1, BL], U32, tag="nfs")
            ucm2s = []
            for r in range(BL):
                nc.gpsimd.sparse_gather(ucpts[r][:], uma[:, r, :],
                                        num_found=nfs[:, r:r + 1])
                # bounce to cm2 layout issued from the pool engine itself:
                # no other engine parks, and it pipelines with gather r+1
                nc.gpsimd.dma_start(
                    out=AP(tensor=ucs_s, offset=r * KCAP,
                           ap=[[16, 16], [1, 16]]), in_=ucpts[r][:])
                ucm2 = pool.tile([P, KCH], F32, tag=f"ucm2_{r}")
                nc.gpsimd.dma_start(
                    out=ucm2[:], in_=AP(tensor=ucs_s, offset=r * KCAP,
                                        ap=[[KCH, P], [1, KCH]]))
                ucm2s.append(ucm2)

            # ---- input triggers: sync gets the early/critical, scalar the rest
            coT_ld = pool.tile([P, C // P, BL], F32, tag="coT_ld")
            dma2(out=coT_ld[:], in_=AP(tensor=coT_d, offset=0,
                                       ap=[[BL, P], [P * BL, C // P], [1, BL]]))
            wcat_ld = pool.tile([P, C // P, 69], F32, tag="wcat_ld")
            dma2(out=wcat_ld[:], in_=AP(tensor=wcat_d, offset=0,
                                        ap=[[69, P], [P * 69, C // P],
                                            [1, 69]]))
            u_b_all = pool.tile([P, BL * N], F32, tag="u_b_all")
            for r in range(BL):
                dma(out=u_b_all[:, r * N:(r + 1) * N],
                    in_=AP(tensor=u_d, offset=r * N, ap=[[0, P], [1, N]]))
            memts = []
            for h in range(2):
                memt2 = pool.tile([P, 2, NCH, W], F16, tag=f"memt_{h}")
                memts.append(memt2)
            dma(out=memts[0][:],
                in_=AP(tensor=mem_d, offset=0,
                       ap=[[NCH * W, P], [N * W, 2], [W, NCH], [1, W]]))

            bcat_sb = pool.tile([BL, 69], F32, tag="bcat")
            dma2(out=bcat_sb[:], in_=bcat_d[:])
            ksqn_sb = pool.tile([BL, KT], F32, tag="ksqn")
            dma2(out=ksqn_sb[:], in_=ksqn_d[:])
            ident_sb = pool.tile([P, P], F32, tag="ident")
            dma2(out=ident_sb[:], in_=ident_d[:])
            slotid_sb = pool.tile([P, KCH], F32, tag="slotid")
            dma2(out=slotid_sb[:], in_=slotid_d[:])
            dma(out=memts[1][:],
                in_=AP(tensor=mem_d, offset=2 * N * W,
                       ap=[[NCH * W, P], [N * W, 2], [W, NCH], [1, W]]))
            vsba = pool.tile([P, BL, NCH + KT - 1], F16, tag="vsba")
            dma(out=vsba[:], in_=AP(tensor=wext_d, offset=0,
                                     ap=[[NCH, P], [N + KT - 1, BL],
                                         [1, NCH + KT - 1]]))

            # ---- DVE consts + fp16 bounces for the phase-A matmuls
            ones1 = pool.tile([1, P], F32, tag="ones1")
            nc.vector.memset(ones1[:], 1.0)
            ones16 = pool.tile([P, 1], F16, tag="ones16")
            nc.vector.memset(ones16[:], 1.0)
            ones_sb = pool.tile([P, 1], F32, tag="ones")
            nc.vector.memset(ones_sb[:], 1.0)
            eps_t = pool.tile([BL, 1], F32, tag="eps")
            nc.vector.memset(eps_t[:], float(EPS))
            coT_sb = pool.tile([P, C // P, BL], F16, tag="coT")
            nc.vector.tensor_copy(coT_sb[:], coT_ld[:])
            wcat_sb = pool.tile([P, C // P, 69], F16, tag="wcat")
            nc.vector.tensor_copy(wcat_sb[:], wcat_ld[:])

            # ---- PE: phase A matmuls first (idle otherwise)
            psA = ppool.tile([BL, 69], F32, tag="psA")
            for k in range(C // P):
                nc.tensor.matmul(psA[:], coT_sb[:, k, :], wcat_sb[:, k, :],
                                 start=(k == 0), stop=(k == C // P - 1))


            # ---- phase A head (deps land ~12us; scalar loads its tables)
            zs = pool.tile([BL, 69], F32, tag="zs")
            nc.vector.tensor_add(zs[:], psA[:], bcat_sb[:])
            z3 = zs[:, W + 1:W + 4]
            z3m = pool.tile([BL, 1], F32, tag="z3m")
            nc.vector.reduce_max(z3m[:], z3, axis=AX.X)
            kt_t = pool.tile([BL, W], F32, tag="kt")
            nc.scalar.activation(kt_t[:], zs[:, 0:W], AF.Tanh)
            # softplus via exp + ln(1+x): no Softplus act-table in this build
            bexp = pool.tile([BL, 1], F32, tag="bexp")
            nc.scalar.activation(bexp[:], zs[:, W:W + 1], AF.Exp)
            beta = pool.tile([BL, 1], F32, tag="beta")
            nc.scalar.activation(beta[:], bexp[:], AF.Ln, bias=1.0)

            # ---------------- phase E tail: masks + PE reduce per row ------
            # Phase-A scraps are interleaved between rows so neither DVE nor
            # the scalar engine ever parks in front of the row pipeline.
            pms, psGs = [], []
            for r in range(BL):
                ucm2 = ucm2s[r]
                # Ltil = ln(clamp(1-u, tiny, 1)); garbage slots contribute 0
                omu = pool.tile([P, KCH], F32, tag=f"omu_{r}")
                nc.vector.tensor_scalar(out=omu[:], in0=ucm2[:], scalar1=1.0,
                                        scalar2=-1.0, op0=ALU.subtract,
                                        op1=ALU.mult)
                omc = pool.tile([P, KCH], F32, tag=f"omc_{r}")
                nc.vector.tensor_scalar(out=omc[:], in0=omu[:], scalar1=1e-30,
                                        scalar2=1.0, op0=ALU.max, op1=ALU.min)
                L32 = pool.tile([P, KCH], F32, tag=f"L32_{r}")
                nc.scalar.activation(L32[:], omc[:], AF.Ln)
                # hw ucode rewrites the whole output tile from its internal
                # scratch: tail slots hold the PREVIOUS call's compacted
                # values. Zero L beyond num_found so they contribute nothing.
                nf_f = pool.tile([1, 1], F32, tag=f"nf_f{r}")
                nc.vector.tensor_copy(nf_f[:], nfs[:, r:r + 1])
                psNf = ppool.tile([P, 1], F32, tag="psNf")
                nc.tensor.matmul(psNf[:], ones1[:], nf_f[:], start=True,
                                 stop=True)
                valid = pool.tile([P, KCH], F32, tag=f"valid_{r}")
                nc.vector.tensor_scalar(out=valid[:], in0=slotid_sb[:],
                                        scalar1=psNf[:], scalar2=None,
                                        op0=ALU.is_lt)
                L32c = pool.tile([P, KCH], F32, tag=f"L32c_{r}")
                nc.vector.tensor_mul(L32c[:], L32[:], valid[:])

                # maskedL: L_slot * [t_p >= u_slot] (is_ge includes the self
                # L term); pair-sum the two slot chunks so PE sees one lhsT
                ml0 = pool.tile([P, N], F16, tag="ml0")
                nc.vector.tensor_scalar(
                    out=ml0[:], in0=u_b_all[:, r * N:(r + 1) * N],
                    scalar1=ucm2[:, 0:1], scalar2=L32c[:, 0:1],
                    op0=ALU.is_ge, op1=ALU.mult)
                ml1 = pool.tile([P, N], F16, tag="ml1")
                nc.vector.tensor_scalar(
                    out=ml1[:], in0=u_b_all[:, r * N:(r + 1) * N],
                    scalar1=ucm2[:, 1:2], scalar2=L32c[:, 1:2],
                    op0=ALU.is_ge, op1=ALU.mult)
                pm = pool.tile([P, N], F16, tag=f"pm{r % 2}")
                nc.vector.tensor_add(pm[:], ml0[:], ml1[:])
                pms.append(pm)

                # G+L in cm layout on psum partitions via maskedL-as-lhsT
                psG = ppool.tile([P, NCH], F32, tag=f"psG{r % 2}")
                for t in range(NCH):
                    nc.tensor.matmul(psG[:, t:t + 1], pm[:, t * P:(t + 1) * P],
                                     ones16[:], start=True, stop=True)
                psGs.append(psG)

                # phase-A scraps fill the gaps between rows
                if r == 0:
                    kb = pool.tile([BL, W], F32, tag="kb")
                    nc.vector.tensor_scalar_mul(kb[:], kt_t[:], beta[:])
                    nz3 = pool.tile([BL, 1], F32, tag="nz3")
                    nc.scalar.mul(nz3[:], z3m[:], -1.0)
                    e3 = pool.tile([BL, 3], F32, tag="e3")
                    nc.scalar.activation(e3[:], z3, AF.Exp, bias=nz3[:])
                elif r == 1:
                    s3 = pool.tile([BL, 1], F32, tag="s3")
                    nc.vector.reduce_sum(s3[:], e3[:], axis=AX.X)
                    r3 = pool.tile([BL, 1], F32, tag="r3")
                    nc.vector.reciprocal(r3[:], s3[:])
                    scr = pool.tile([BL, 1], F32, tag="scr")
                    nc.vector.tensor_sub(scr[:], e3[:, 2:3], e3[:, 0:1])
                    sc = pool.tile([BL, 1], F32, tag="sc")
                    nc.vector.tensor_mul(sc[:], scr[:], r3[:])
                    sq = pool.tile([BL, 1], F32, tag="sq")
                    nc.scalar.square(sq[:], sc[:])
                    tau = pool.tile([BL, 1], F32, tag="tau")
                    nc.scalar.activation(tau[:], sq[:], AF.Identity,
                                         bias=eps_t[:], scale=2.0)
                    wgt = pool.tile([BL, 1], F32, tag="wgt")
                    nc.scalar.activation(wgt[:], zs[:, W + 4:W + 5],
                                         AF.Sigmoid)
                    wh = pool.tile([BL, 1], F32, tag="wh")
                    nc.scalar.mul(wh[:], wgt[:], 0.5)
                    dma2(out=kb_s[:].rearrange("(r w) -> r w", r=BL),
                         in_=kb[:])
                    dma2(out=wh_s[:].rearrange("(r o) -> r o", r=BL),
                         in_=wh[:])
                elif r == 2:
                    rtau = pool.tile([BL, 1], F32, tag="rtau")
                    nc.vector.reciprocal(rtau[:], tau[:])
                    garg = pool.tile([BL, KT], F32, tag="garg")
                    nc.vector.tensor_scalar_mul(garg[:], ksqn_sb[:], rtau[:])
                    g_t = pool.tile([BL, KT], F32, tag="g")
                    nc.scalar.activation(g_t[:], garg[:], AF.Exp)
                else:
                    S_t = pool.tile([BL, 1], F32, tag="S")
                    nc.vector.reduce_sum(S_t[:], g_t[:], axis=AX.X)
                    Se = pool.tile([BL, 1], F32, tag="Se")
                    nc.scalar.activation(Se[:], S_t[:], AF.Identity,
                                         bias=eps_t[:])
                    rS = pool.tile([BL, 1], F32, tag="rS")
                    nc.vector.reciprocal(rS[:], Se[:])
                    gn = pool.tile([BL, KT], F32, tag="gn")
                    nc.vector.tensor_scalar_mul(gn[:], g_t[:], rS[:])
                    dma2(out=gn_s[:].rearrange("(r j) -> r j", r=BL),
                         in_=gn[:])
                    kb_ba = pool.tile([P, BL, W], F32, tag="kb_ba")
                    dma2(out=kb_ba[:], in_=AP(tensor=kb_s, offset=0,
                                              ap=[[0, P], [1, BL * W]]))
                    gnb = pool.tile([P, BL, KT], F32, tag="gnb")
                    dma2(out=gnb[:], in_=AP(tensor=gn_s, offset=0,
                                            ap=[[0, P], [KT, BL], [1, KT]]))
                    whb = pool.tile([P, BL], F32, tag="whb")
                    dma2(out=whb[:], in_=AP(tensor=wh_s, offset=0,
                                            ap=[[0, P], [1, BL]]))

            # ---- per-row epilogue: exp, transpose, rm bounce
            al_rms = []
            for r in range(BL):
                al_cm = pool.tile([P, NCH], F32, tag=f"alcm_{r % 2}")
                nc.scalar.activation(al_cm[:], psGs[r][:], AF.Exp)
                # cm -> rm via PE transpose + DRAM round-trip
                psT = ppool.tile([NCH, P], F32, tag=f"psT{r % 2}")
                nc.tensor.transpose(psT[:], al_cm[:], ident_sb[:])
                alT = pool.tile([NCH, P], F32, tag=f"alT_{r % 2}")
                nc.scalar.copy(alT[:], psT[:])
                dma(out=AP(tensor=o_al, offset=r * N,
                           ap=[[P, NCH], [1, P]]), in_=alT[:])
                al_rm = pool.tile([P, NCH], F32, tag=f"alrm{r}")
                dma(out=al_rm[:], in_=AP(tensor=o_al, offset=r * N,
                                         ap=[[NCH, P], [1, NCH]]))
                al_rms.append(al_rm)

            # ---------------- phase B: sim = mem . (k*beta), fp16, rm ------
            sim_all = pool.tile([P, BL, NCH], F32, tag="sim_all")
            kb16a = pool.tile([P, BL, W], F16, tag="kb16a")
            nc.vector.tensor_copy(kb16a[:], kb_ba[:])
            for r in range(BL):
                smul = pool.tile([P, NCH, W], F16, tag=f"smul{r % 2}")
                nc.vector.tensor_mul(
                    smul[:], memts[r // 2][:, r % 2, :, :],
                    kb16a[:, r, :].unsqueeze(1).broadcast_to([P, NCH, W]))
                nc.vector.tensor_reduce(sim_all[:, r, :], smul[:], axis=AX.X,
                                        op=ALU.add)

            # ---------------- phase C: content softmax (no max-shift) -----
            e_cm = pool.tile([P, BL, NCH], F32, tag="e_cm")
            nc.scalar.activation(e_cm[:], sim_all[:], AF.Exp)
            esum = pool.tile([P, BL], F32, tag="esum")
            nc.vector.tensor_reduce(esum[:], e_cm[:], axis=AX.X, op=ALU.add)
            psC = ppool.tile([1, BL], F32, tag="psC")
            nc.tensor.matmul(psC[:], ones_sb[:], esum[:], start=True, stop=True)
            rCs = pool.tile([1, BL], F32, tag="rCs")
            nc.vector.reciprocal(rCs[:], psC[:])
            rsb = ppool.tile([P, BL], F32, tag="rsb")
            nc.tensor.matmul(rsb[:], ones1[:], rCs[:], start=True, stop=True)

            # ---------------- phase D: directional (16-tap), fp16, rm ------
            dw_all = pool.tile([P, BL, NCH], F32, tag="dw_all")
            gnb16 = pool.tile([P, BL, KT], F16, tag="gnb16")
            nc.vector.tensor_copy(gnb16[:], gnb[:])
            for r in range(BL):
                dmul = pool.tile([P, NCH, KT], F16, tag=f"dmul{r}")
                nc.vector.tensor_mul(
                    dmul[:], _win(vsba[:, r, :], [[1, NCH], [1, KT]]),
                    gnb16[:, r:r + 1, :].broadcast_to([P, NCH, KT]))
                nc.vector.tensor_reduce(dw_all[:, r, :], dmul[:], axis=AX.X,
                                        op=ALU.add)

            # ---------------- phase F: combine + fused stores (rm) ---------
            rm_all = lambda d: AP(tensor=d, offset=0,
                                  ap=[[NCH, P], [N, BL], [1, NCH]])
            dma2(out=rm_all(o_dw), in_=dw_all[:])
            cw_all = pool.tile([P, BL, NCH], F32, tag="cw_all")
            ww_all = pool.tile([P, BL, NCH], F32, tag="ww_all")
            for r in range(BL):
                nc.vector.tensor_scalar_mul(cw_all[:, r, :], e_cm[:, r, :],
                                            rsb[:, r:r + 1])
                dwal = pool.tile([P, NCH], F32, tag=f"dwal{r}")
                nc.vector.tensor_mul(dwal[:], dw_all[:, r, :], al_rms[r][:])
                tsum = pool.tile([P, NCH], F32, tag=f"tsum{r}")
                nc.vector.tensor_add(tsum[:], cw_all[:, r, :], dwal[:])
                nc.vector.tensor_scalar_mul(ww_all[:, r, :], tsum[:],
                                            whb[:, r:r + 1])
            dma(out=rm_all(o_cw), in_=cw_all[:])
            dma2(out=rm_all(o_ww), in_=ww_all[:])

    _split_waits(nc)
    lower_extended_insts(nc)
    return nc


def _dedup_rows(u):
    """Make every row's values unique by bumping later duplicates up in
    2^-23 quanta (the sparse_gather ucode's fixed-point grid). Matches the
    reference's argsort lex order to ~1e-7."""
    q = np.float32(2.0 ** -23)
    u = u.copy()
    for r in range(u.shape[0]):
        row = u[r]
        for _ in range(8):
            vals, counts = np.unique(row, return_counts=True)
            dups = vals[counts > 1]
            if dups.size == 0:
                break
            for v in dups:
                idx = np.flatnonzero(row == v)[1:]
                for j, p in enumerate(idx):
                    row[p] = v + np.float32(j + 1) * q
    return u


def _host_prep(inputs):
    co = np.ascontiguousarray(inputs["controller_output"], dtype=np.float32)
    prw = np.ascontiguousarray(inputs["prev_read_weights"], dtype=np.float32)
    memory = np.ascontiguousarray(inputs["memory"], dtype=np.float32)
    usage = _dedup_rows(np.asarray(inputs["usage"], dtype=np.float32))

    cnt = (usage < UT).sum(axis=1)
    assert cnt.max() <= KCAP, f"compaction overflow: {cnt.max()} > {KCAP}"

    wcat = np.concatenate([np.asarray(inputs["Wk"]), np.asarray(inputs["Wb"]),
                           np.asarray(inputs["Ws"]), np.asarray(inputs["Wg"])],
                          axis=0).T  # [C, 69]
    wcat = np.ascontiguousarray(wcat, dtype=np.float32)
    bcat = np.concatenate([np.asarray(inputs["bk"]), np.asarray(inputs["bb"]),
                           np.asarray(inputs["bs"]),
                           np.asarray(inputs["bg"])]).astype(np.float32)
    bcat_rep = np.ascontiguousarray(np.broadcast_to(bcat, (BL, 69)))

    # v[m] = w[(m-1024) % N]; extended with KT-1 wrap elements
    v = np.concatenate([prw[:, N // 2:], prw[:, :N // 2]], axis=1)
    wext = np.ascontiguousarray(
        np.concatenate([v, v[:, :KT - 1]], axis=1).astype(np.float16))

    ident = np.eye(P, dtype=np.float32)
    ksqn = np.ascontiguousarray(np.broadcast_to(
        -(np.arange(KT, dtype=np.float32) ** 2), (BL, KT)), dtype=np.float32)
    # slot order: gather writes slot i at (partition i%16, free i//16) of the
    # [16, KCAP/16] tile; the DRAM bounce stores linear j = p*16+f and the
    # cm2 reload maps j = 2*p2+k. slotid = gather index of each (p2, k).
    j = (np.arange(P, dtype=np.int64)[:, None] * KCH
         + np.arange(KCH, dtype=np.int64)[None, :])
    slotid = np.ascontiguousarray(
        ((j % 16) * (KCAP // 16) + j // 16).astype(np.float32))

    in_maps = []
    for cidx in range(NCORES):
        rows = slice(cidx * BL, (cidx + 1) * BL)
        in_maps.append({
            "mem": np.ascontiguousarray(memory[rows].astype(np.float16)),
            "coT": np.ascontiguousarray(co[rows].T),
            "wcat": wcat,
            "bcat": bcat_rep,
            "wext": np.ascontiguousarray(wext[rows]),
            "u": np.ascontiguousarray(usage[rows]),
            "ksqn": ksqn,
            "ident": ident,
            "slotid": slotid,
        })
    return in_maps


def kernel(**inputs):
    return _run(inputs, trace=False)[0]


def _run(inputs, trace=False):
    from concourse.bass_utils import run_bass_kernel_spmd

    if "nc" not in _CACHE:
        _CACHE["nc"] = _build()
    nc = _CACHE["nc"]

    in_maps = _host_prep(inputs)
    res = run_bass_kernel_spmd(nc, in_maps, core_ids=list(range(NCORES)),
                               trace=trace)

    ww = np.concatenate([res.results[i]["o_ww"] for i in range(NCORES)], axis=0)
    cw = np.concatenate([res.results[i]["o_cw"] for i in range(NCORES)], axis=0)
    dw = np.concatenate([res.results[i]["o_dw"] for i in range(NCORES)], axis=0)
    al = np.concatenate([res.results[i]["o_al"] for i in range(NCORES)], axis=0)
    out = (ww.astype(np.float32), cw.astype(np.float32),
           dw.astype(np.float32), al.astype(np.float32))
    return out, res


# revision 21
# speedup vs baseline: 1.0415x; 1.0185x over previous
"""DNC addressing kernel for Trainium2, 8 NeuronCores, batch-sharded.

Math reformulations vs the reference (numerically validated):
  * directional: the [B,N,N] shift kernel is circulant with row-constant
    normalization; dw[m] = sum_j gn[j] * w[(m-1024+j) % N] with j <= 15
    (Gaussian taps decay below f32 eps past j=6 even at max |sc|).
  * allocation: alloc[p] = exp(G_p + L_p), L = log1p(-u),
    G_p = sum over q with u_q < u_p of L_q (host nudge makes u unique).
    Only the ~210 smallest-usage entries per row give alloc above ~1e-6
    (tolerance is 2e-2), so the smallest entries (u < T) are compacted
    with the gpsimd sparse_gather ucode (exact for u that are multiples
    of 2^-23, which the inputs are), compared against all 2048 thresholds
    with DVE tensor_scalar is_ge masks (fp16 {0,1}), and reduced with
    mask-as-lhsT fp16 PE matmuls giving exp-ready G+L directly in cm
    layout on 128 psum partitions. Thresholds >= T fall out as
    exp(G_total) with error < 3e-4.

Layouts: "rm" means n = p*16 + c (contiguous 64B runs per partition, fast
DMA), "cm" means n = c*128 + p (the mask column order). alloc converts
cm->rm via PE transpose + a DRAM round-trip through its own output.
"""

import sys

for _p in ("/opt/trn_rl_repo", "/root/.axon_site/_ro/trn_rl_repo"):
    if _p not in sys.path:
        sys.path.append(_p)

import numpy as np

import concourse.bass as bass
import concourse.mybir as mybir
from bass_rust import AP
from concourse.tile import TileContext
from concourse import library_config
from concourse.library_overlay import lower_extended_insts

F32 = mybir.dt.float32
F16 = mybir.dt.float16
U32 = mybir.dt.uint32
AF = mybir.ActivationFunctionType
ALU = mybir.AluOpType
AX = mybir.AxisListType

NCORES = 8
B, N, W, C = 32, 2048, 64, 1024
BL = B // NCORES          # 4 rows per core
P = 128                   # partitions
NCH = N // P              # 16 cm chunks
KT = 16                   # directional taps
EPS = 1e-8

UT = 0.09                 # usage compaction cutoff
KCAP = 256                # compacted slot capacity (16x16 tile)
KCH = KCAP // P           # 2 element chunks of 128 slots

_CACHE = {}


def _split_waits(nc, cap=1):
    """Walrus codegen rejects instructions with more than ~1 semaphore wait
    (PE load-weights fails at 2). Hoist excess waits onto same-engine NOPs
    inserted just before the instruction."""
    import bass_rust

    wid = [0]
    for f in nc.m.functions:
        for blk in f.blocks:
            new = []
            for inst in blk.instructions:
                si = inst.sync_info
                waits = list(si.on_wait) if si is not None and si.on_wait else []
                if len(waits) > cap:
                    keep = waits[-cap:]
                    extra = waits[:-cap]
                    for i in range(0, len(extra), cap):
                        nop = bass_rust.InstNoOp(
                            name=f"WNOP-{wid[0]}", ins=[], outs=[])
                        wid[0] += 1
                        nop.engine = inst.engine
                        nop.sync_info = mybir.SyncInfo(
                            on_wait=extra[i:i + cap], on_update=[])
                        new.append(nop)
                    inst.sync_info = mybir.SyncInfo(
                        on_wait=keep, on_update=si.on_update)
                new.append(inst)
            blk.instructions[:] = new


def _win(ap, dims):
    """Raw windowed view of an SBUF tile AP: keep partition dim, replace the
    free dims (overlapping windows allowed)."""
    return AP(tensor=ap.tensor, offset=ap.offset, ap=[ap.ap[0]] + dims)


def _build():
    nc = bass.Bass()

    mem_d = nc.dram_tensor("mem", [BL, N, W], F16, kind="ExternalInput")
    coT_d = nc.dram_tensor("coT", [P, C // P, BL], F32, kind="ExternalInput")
    wcat_d = nc.dram_tensor("wcat", [P, C // P, 69], F32, kind="ExternalInput")
    bcat_d = nc.dram_tensor("bcat", [BL, 69], F32, kind="ExternalInput")
    wext_d = nc.dram_tensor("wext", [BL, N + KT - 1], F16, kind="ExternalInput")
    u_d = nc.dram_tensor("u", [BL, N], F32, kind="ExternalInput")
    ksqn_d = nc.dram_tensor("ksqn", [BL, KT], F32, kind="ExternalInput")
    ident_d = nc.dram_tensor("ident", [P, P], F32, kind="ExternalInput")
    slotid_d = nc.dram_tensor("slotid", [P, KCH], F32, kind="ExternalInput")

    o_ww = nc.dram_tensor("o_ww", [BL, N], F32, kind="ExternalOutput")
    o_cw = nc.dram_tensor("o_cw", [BL, N], F32, kind="ExternalOutput")
    o_dw = nc.dram_tensor("o_dw", [BL, N], F32, kind="ExternalOutput")
    o_al = nc.dram_tensor("o_al", [BL, N], F32, kind="ExternalOutput")

    kb_s = nc.dram_tensor("kb_s", [BL * W], F32, kind="Internal")
    gn_s = nc.dram_tensor("gn_s", [BL * KT], F32, kind="Internal")
    wh_s = nc.dram_tensor("wh_s", [BL], F32, kind="Internal")
    ucs_s = nc.dram_tensor("ucs_s", [BL * KCAP], F32, kind="Internal")

    with TileContext(nc) as tc:
        with tc.tile_pool(name="sb", bufs=1) as pool, \
             tc.tile_pool(name="ps", bufs=1, space="PSUM") as ppool:

            dma = nc.sync.dma_start      # HWDGE engine 1
            dma2 = nc.scalar.dma_start   # HWDGE engine 2

            nc.gpsimd.load_library(library_config.sparse_gather)

            # ---------------- phase E head: compaction of small usage -----
            # u16 wrap order is arbitrary (values only), so use the
            # DMA-friendly p-major mapping; one fused load for all rows.
            # Everything the gathers need is emitted FIRST on each engine.
            u16a = pool.tile([16, BL, P], F32, tag="u16a")
            dma2(out=u16a[:], in_=AP(tensor=u_d, offset=0,
                                     ap=[[P, 16], [N, BL], [1, P]]))

            # um = u - 2*(u >= T): keeps u<T, maps the rest negative
            m2a = pool.tile([16, BL, P], F32, tag="m2a")
            nc.vector.tensor_scalar(out=m2a[:], in0=u16a[:], scalar1=UT,
                                    scalar2=-2.0, op0=ALU.is_ge, op1=ALU.mult)
            uma = pool.tile([16, BL, P], F32, tag="uma")
            nc.vector.tensor_add(uma[:], m2a[:], u16a[:])
            ucpts = []
            for r in range(BL):
                ucpt = pool.tile([16, KCAP // 16], F32, tag=f"ucpt_{r}")
                nc.vector.memset(ucpt[:], 0.5)  # hw ucode rewrites the tail
                ucpts.append(ucpt)

            nfs = pool.tile([1, BL], U32, tag="nfs")
            ucm2s = []
            for r in range(BL):
                nc.gpsimd.sparse_gather(ucpts[r][:], uma[:, r, :],
                                        num_found=nfs[:, r:r + 1])
                # bounce to cm2 layout issued from the pool engine itself:
                # no other engine parks, and it pipelines with gather r+1
                nc.gpsimd.dma_start(
                    out=AP(tensor=ucs_s, offset=r * KCAP,
                           ap=[[16, 16], [1, 16]]), in_=ucpts[r][:])
                ucm2 = pool.tile([P, KCH], F32, tag=f"ucm2_{r}")
                nc.gpsimd.dma_start(
                    out=ucm2[:], in_=AP(tensor=ucs_s, offset=r * KCAP,
                                        ap=[[KCH, P], [1, KCH]]))
                ucm2s.append(ucm2)

            # ---- input triggers: sync gets the early/critical, scalar the rest
            coT_ld = pool.tile([P, C // P, BL], F32, tag="coT_ld")
            dma2(out=coT_ld[:], in_=AP(tensor=coT_d, offset=0,
                                       ap=[[C // P * BL, P], [1, C // P * BL]]))
            wcat_ld = pool.tile([P, C // P, 69], F32, tag="wcat_ld")
            dma2(out=wcat_ld[:], in_=AP(tensor=wcat_d, offset=0,
                                        ap=[[C // P * 69, P],
                                            [1, C // P * 69]]))
            u_b_all = pool.tile([P, BL * N], F32, tag="u_b_all")
            memts = []
            for h in range(2):
                memt2 = pool.tile([P, 2, NCH, W], F16, tag=f"memt_{h}")
                memts.append(memt2)
            dma(out=u_b_all[:, 0:N],
                in_=AP(tensor=u_d, offset=0, ap=[[0, P], [1, N]]))
            dma(out=memts[0][:],
                in_=AP(tensor=mem_d, offset=0,
                       ap=[[NCH * W, P], [N * W, 2], [W, NCH], [1, W]]))
            dma(out=u_b_all[:, N:2 * N],
                in_=AP(tensor=u_d, offset=N, ap=[[0, P], [1, N]]))

            bcat_sb = pool.tile([BL, 69], F32, tag="bcat")
            dma2(out=bcat_sb[:], in_=bcat_d[:])
            ksqn_sb = pool.tile([BL, KT], F32, tag="ksqn")
            dma2(out=ksqn_sb[:], in_=ksqn_d[:])
            ident_sb = pool.tile([P, P], F32, tag="ident")
            dma2(out=ident_sb[:], in_=ident_d[:])
            slotid_sb = pool.tile([P, KCH], F32, tag="slotid")
            dma2(out=slotid_sb[:], in_=slotid_d[:])
            dma(out=memts[1][:],
                in_=AP(tensor=mem_d, offset=2 * N * W,
                       ap=[[NCH * W, P], [N * W, 2], [W, NCH], [1, W]]))
            for r in (2, 3):
                dma(out=u_b_all[:, r * N:(r + 1) * N],
                    in_=AP(tensor=u_d, offset=r * N, ap=[[0, P], [1, N]]))
            vsba = pool.tile([P, BL, NCH + KT - 1], F16, tag="vsba")
            dma(out=vsba[:], in_=AP(tensor=wext_d, offset=0,
                                     ap=[[NCH, P], [N + KT - 1, BL],
                                         [1, NCH + KT - 1]]))

            # ---- DVE consts + fp16 bounces for the phase-A matmuls
            ones1 = pool.tile([1, P], F32, tag="ones1")
            nc.vector.memset(ones1[:], 1.0)
            ones16 = pool.tile([P, 1], F16, tag="ones16")
            nc.vector.memset(ones16[:], 1.0)
            ones_sb = pool.tile([P, 1], F32, tag="ones")
            nc.vector.memset(ones_sb[:], 1.0)
            eps_t = pool.tile([BL, 1], F32, tag="eps")
            nc.vector.memset(eps_t[:], float(EPS))
            coT_sb = pool.tile([P, C // P, BL], F16, tag="coT")
            nc.vector.tensor_copy(coT_sb[:], coT_ld[:])
            wcat_sb = pool.tile([P, C // P, 69], F16, tag="wcat")
            nc.vector.tensor_copy(wcat_sb[:], wcat_ld[:])

            # ---- PE: phase A matmuls first (idle otherwise)
            psA = ppool.tile([BL, 69], F32, tag="psA")
            for k in range(C // P):
                nc.tensor.matmul(psA[:], coT_sb[:, k, :], wcat_sb[:, k, :],
                                 start=(k == 0), stop=(k == C // P - 1))


            # ---- phase A head (deps land ~12us; scalar loads its tables)
            zs = pool.tile([BL, 69], F32, tag="zs")
            nc.vector.tensor_add(zs[:], psA[:], bcat_sb[:])
            z3 = zs[:, W + 1:W + 4]
            z3m = pool.tile([BL, 1], F32, tag="z3m")
            nc.vector.reduce_max(z3m[:], z3, axis=AX.X)
            kt_t = pool.tile([BL, W], F32, tag="kt")
            nc.scalar.activation(kt_t[:], zs[:, 0:W], AF.Tanh)
            # softplus via exp + ln(1+x): no Softplus act-table in this build
            bexp = pool.tile([BL, 1], F32, tag="bexp")
            nc.scalar.activation(bexp[:], zs[:, W:W + 1], AF.Exp)
            beta = pool.tile([BL, 1], F32, tag="beta")
            nc.scalar.activation(beta[:], bexp[:], AF.Ln, bias=1.0)

            # ---- phase A scraps whose deps land early + B rows 0/1 fill
            # the DVE idle window while the gathers run
            kb = pool.tile([BL, W], F32, tag="kb")
            nc.vector.tensor_scalar_mul(kb[:], kt_t[:], beta[:])
            dma2(out=kb_s[:].rearrange("(r w) -> r w", r=BL), in_=kb[:])
            kb_ba = pool.tile([P, BL, W], F32, tag="kb_ba")
            dma2(out=kb_ba[:], in_=AP(tensor=kb_s, offset=0,
                                      ap=[[0, P], [1, BL * W]]))
            kb16a = pool.tile([P, BL, W], F16, tag="kb16a")
            nc.vector.tensor_copy(kb16a[:], kb_ba[:])
            sim_all = pool.tile([P, BL, NCH], F32, tag="sim_all")
            for r in (0, 1):
                smul = pool.tile([P, NCH, W], F16, tag=f"smul{r % 2}")
                nc.vector.tensor_mul(
                    smul[:], memts[r // 2][:, r % 2, :, :],
                    kb16a[:, r, :].unsqueeze(1).broadcast_to([P, NCH, W]))
                nc.vector.tensor_reduce(sim_all[:, r, :], smul[:], axis=AX.X,
                                        op=ALU.add)

            # ---------------- phase E tail: masks + PE reduce per row ------
            # Phase-A scraps are interleaved between rows so neither DVE nor
            # the scalar engine ever parks in front of the row pipeline.
            pms, psGs = [], []
            for r in range(BL):
                ucm2 = ucm2s[r]
                # Ltil = ln(clamp(1-u, tiny, 1)); garbage slots contribute 0
                omu = pool.tile([P, KCH], F32, tag=f"omu_{r}")
                nc.vector.tensor_scalar(out=omu[:], in0=ucm2[:], scalar1=1.0,
                                        scalar2=-1.0, op0=ALU.subtract,
                                        op1=ALU.mult)
                omc = pool.tile([P, KCH], F32, tag=f"omc_{r}")
                nc.vector.tensor_scalar(out=omc[:], in0=omu[:], scalar1=1e-30,
                                        scalar2=1.0, op0=ALU.max, op1=ALU.min)
                L32 = pool.tile([P, KCH], F32, tag=f"L32_{r}")
                nc.scalar.activation(L32[:], omc[:], AF.Ln)
                # hw ucode rewrites the whole output tile from its internal
                # scratch: tail slots hold the PREVIOUS call's compacted
                # values. Zero L beyond num_found so they contribute nothing.
                nf_f = pool.tile([1, 1], F32, tag=f"nf_f{r}")
                nc.vector.tensor_copy(nf_f[:], nfs[:, r:r + 1])
                psNf = ppool.tile([P, 1], F32, tag="psNf")
                nc.tensor.matmul(psNf[:], ones1[:], nf_f[:], start=True,
                                 stop=True)
                valid = pool.tile([P, KCH], F32, tag=f"valid_{r}")
                nc.vector.tensor_scalar(out=valid[:], in0=slotid_sb[:],
                                        scalar1=psNf[:], scalar2=None,
                                        op0=ALU.is_lt)
                L32c = pool.tile([P, KCH], F32, tag=f"L32c_{r}")
                nc.vector.tensor_mul(L32c[:], L32[:], valid[:])

                # maskedL: L_slot * [t_p >= u_slot] (is_ge includes the self
                # L term); pair-sum the two slot chunks so PE sees one lhsT
                ml0 = pool.tile([P, N], F16, tag="ml0")
                nc.vector.tensor_scalar(
                    out=ml0[:], in0=u_b_all[:, r * N:(r + 1) * N],
                    scalar1=ucm2[:, 0:1], scalar2=L32c[:, 0:1],
                    op0=ALU.is_ge, op1=ALU.mult)
                ml1 = pool.tile([P, N], F16, tag="ml1")
                nc.vector.tensor_scalar(
                    out=ml1[:], in0=u_b_all[:, r * N:(r + 1) * N],
                    scalar1=ucm2[:, 1:2], scalar2=L32c[:, 1:2],
                    op0=ALU.is_ge, op1=ALU.mult)
                pm = pool.tile([P, N], F16, tag=f"pm{r % 2}")
                nc.vector.tensor_add(pm[:], ml0[:], ml1[:])
                pms.append(pm)

                # G+L in cm layout on psum partitions via maskedL-as-lhsT
                psG = ppool.tile([P, NCH], F32, tag=f"psG{r % 2}")
                for t in range(NCH):
                    nc.tensor.matmul(psG[:, t:t + 1], pm[:, t * P:(t + 1) * P],
                                     ones16[:], start=True, stop=True)
                psGs.append(psG)

                # phase-A scraps fill the gaps between rows
                if r == 0:
                    nz3 = pool.tile([BL, 1], F32, tag="nz3")
                    nc.scalar.mul(nz3[:], z3m[:], -1.0)
                    e3 = pool.tile([BL, 3], F32, tag="e3")
                    nc.scalar.activation(e3[:], z3, AF.Exp, bias=nz3[:])
                elif r == 1:
                    s3 = pool.tile([BL, 1], F32, tag="s3")
                    nc.vector.reduce_sum(s3[:], e3[:], axis=AX.X)
                    r3 = pool.tile([BL, 1], F32, tag="r3")
                    nc.vector.reciprocal(r3[:], s3[:])
                    scr = pool.tile([BL, 1], F32, tag="scr")
                    nc.vector.tensor_sub(scr[:], e3[:, 2:3], e3[:, 0:1])
                    sc = pool.tile([BL, 1], F32, tag="sc")
                    nc.vector.tensor_mul(sc[:], scr[:], r3[:])
                    sq = pool.tile([BL, 1], F32, tag="sq")
                    nc.scalar.square(sq[:], sc[:])
                    tau = pool.tile([BL, 1], F32, tag="tau")
                    nc.scalar.activation(tau[:], sq[:], AF.Identity,
                                         bias=eps_t[:], scale=2.0)
                    wgt = pool.tile([BL, 1], F32, tag="wgt")
                    nc.scalar.activation(wgt[:], zs[:, W + 4:W + 5],
                                         AF.Sigmoid)
                    wh = pool.tile([BL, 1], F32, tag="wh")
                    nc.scalar.mul(wh[:], wgt[:], 0.5)
                    dma2(out=wh_s[:].rearrange("(r o) -> r o", r=BL),
                         in_=wh[:])
                elif r == 2:
                    rtau = pool.tile([BL, 1], F32, tag="rtau")
                    nc.vector.reciprocal(rtau[:], tau[:])
                    garg = pool.tile([BL, KT], F32, tag="garg")
                    nc.vector.tensor_scalar_mul(garg[:], ksqn_sb[:], rtau[:])
                    g_t = pool.tile([BL, KT], F32, tag="g")
                    nc.scalar.activation(g_t[:], garg[:], AF.Exp)
                else:
                    S_t = pool.tile([BL, 1], F32, tag="S")
                    nc.vector.reduce_sum(S_t[:], g_t[:], axis=AX.X)
                    Se = pool.tile([BL, 1], F32, tag="Se")
                    nc.scalar.activation(Se[:], S_t[:], AF.Identity,
                                         bias=eps_t[:])
                    rS = pool.tile([BL, 1], F32, tag="rS")
                    nc.vector.reciprocal(rS[:], Se[:])
                    gn = pool.tile([BL, KT], F32, tag="gn")
                    nc.vector.tensor_scalar_mul(gn[:], g_t[:], rS[:])
                    dma2(out=gn_s[:].rearrange("(r j) -> r j", r=BL),
                         in_=gn[:])
                    kb_ba = pool.tile([P, BL, W], F32, tag="kb_ba")
                    dma2(out=kb_ba[:], in_=AP(tensor=kb_s, offset=0,
                                              ap=[[0, P], [1, BL * W]]))
                    gnb = pool.tile([P, BL, KT], F32, tag="gnb")
                    dma2(out=gnb[:], in_=AP(tensor=gn_s, offset=0,
                                            ap=[[0, P], [KT, BL], [1, KT]]))
                    whb = pool.tile([P, BL], F32, tag="whb")
                    dma2(out=whb[:], in_=AP(tensor=wh_s, offset=0,
                                            ap=[[0, P], [1, BL]]))

            # ---- per-row epilogue: exp, transpose, rm bounce
            al_rms = []
            for r in range(BL):
                al_cm = pool.tile([P, NCH], F32, tag=f"alcm_{r % 2}")
                nc.scalar.activation(al_cm[:], psGs[r][:], AF.Exp)
                # cm -> rm via PE transpose + DRAM round-trip
                psT = ppool.tile([NCH, P], F32, tag=f"psT{r % 2}")
                nc.tensor.transpose(psT[:], al_cm[:], ident_sb[:])
                alT = pool.tile([NCH, P], F32, tag=f"alT_{r % 2}")
                nc.scalar.copy(alT[:], psT[:])
                dma(out=AP(tensor=o_al, offset=r * N,
                           ap=[[P, NCH], [1, P]]), in_=alT[:])
                al_rm = pool.tile([P, NCH], F32, tag=f"alrm{r}")
                dma(out=al_rm[:], in_=AP(tensor=o_al, offset=r * N,
                                         ap=[[NCH, P], [1, NCH]]))
                al_rms.append(al_rm)

            # ---------------- phase B rows 2/3 (0/1 ran before the masks) --
            for r in (2, 3):
                smul = pool.tile([P, NCH, W], F16, tag=f"smul{r % 2}")
                nc.vector.tensor_mul(
                    smul[:], memts[r // 2][:, r % 2, :, :],
                    kb16a[:, r, :].unsqueeze(1).broadcast_to([P, NCH, W]))
                nc.vector.tensor_reduce(sim_all[:, r, :], smul[:], axis=AX.X,
                                        op=ALU.add)

            # ---------------- phase C: content softmax (no max-shift) -----
            e_cm = pool.tile([P, BL, NCH], F32, tag="e_cm")
            nc.scalar.activation(e_cm[:], sim_all[:], AF.Exp)
            esum = pool.tile([P, BL], F32, tag="esum")
            nc.vector.tensor_reduce(esum[:], e_cm[:], axis=AX.X, op=ALU.add)
            psC = ppool.tile([1, BL], F32, tag="psC")
            nc.tensor.matmul(psC[:], ones_sb[:], esum[:], start=True, stop=True)
            rCs = pool.tile([1, BL], F32, tag="rCs")
            nc.vector.reciprocal(rCs[:], psC[:])
            rsb = ppool.tile([P, BL], F32, tag="rsb")
            nc.tensor.matmul(rsb[:], ones1[:], rCs[:], start=True, stop=True)

            # ---------------- phase D: directional (16-tap), fp16, rm ------
            dw_all = pool.tile([P, BL, NCH], F32, tag="dw_all")
            gnb16 = pool.tile([P, BL, KT], F16, tag="gnb16")
            nc.vector.tensor_copy(gnb16[:], gnb[:])
            for r in range(BL):
                dmul = pool.tile([P, NCH, KT], F16, tag=f"dmul{r}")
                nc.vector.tensor_mul(
                    dmul[:], _win(vsba[:, r, :], [[1, NCH], [1, KT]]),
                    gnb16[:, r:r + 1, :].broadcast_to([P, NCH, KT]))
                nc.vector.tensor_reduce(dw_all[:, r, :], dmul[:], axis=AX.X,
                                        op=ALU.add)

            # ---------------- phase F: combine + fused stores (rm) ---------
            rm_all = lambda d: AP(tensor=d, offset=0,
                                  ap=[[NCH, P], [N, BL], [1, NCH]])
            dma2(out=rm_all(o_dw), in_=dw_all[:])
            cw_all = pool.tile([P, BL, NCH], F32, tag="cw_all")
            ww_all = pool.tile([P, BL, NCH], F32, tag="ww_all")
            for r in range(BL):
                nc.vector.tensor_scalar_mul(cw_all[:, r, :], e_cm[:, r, :],
                                            rsb[:, r:r + 1])
                dwal = pool.tile([P, NCH], F32, tag=f"dwal{r}")
                nc.vector.tensor_mul(dwal[:], dw_all[:, r, :], al_rms[r][:])
                tsum = pool.tile([P, NCH], F32, tag=f"tsum{r}")
                nc.vector.tensor_add(tsum[:], cw_all[:, r, :], dwal[:])
                nc.vector.tensor_scalar_mul(ww_all[:, r, :], tsum[:],
                                            whb[:, r:r + 1])
            dma(out=rm_all(o_cw), in_=cw_all[:])
            dma2(out=rm_all(o_ww), in_=ww_all[:])

    _split_waits(nc)
    lower_extended_insts(nc)
    return nc


def _dedup_rows(u):
    """Make every row's values unique by bumping later duplicates up in
    2^-23 quanta (the sparse_gather ucode's fixed-point grid). Matches the
    reference's argsort lex order to ~1e-7."""
    q = np.float32(2.0 ** -23)
    u = u.copy()
    for r in range(u.shape[0]):
        row = u[r]
        for _ in range(8):
            vals, counts = np.unique(row, return_counts=True)
            dups = vals[counts > 1]
            if dups.size == 0:
                break
            for v in dups:
                idx = np.flatnonzero(row == v)[1:]
                for j, p in enumerate(idx):
                    row[p] = v + np.float32(j + 1) * q
    return u


def _host_prep(inputs):
    co = np.ascontiguousarray(inputs["controller_output"], dtype=np.float32)
    prw = np.ascontiguousarray(inputs["prev_read_weights"], dtype=np.float32)
    memory = np.ascontiguousarray(inputs["memory"], dtype=np.float32)
    usage = _dedup_rows(np.asarray(inputs["usage"], dtype=np.float32))

    cnt = (usage < UT).sum(axis=1)
    assert cnt.max() <= KCAP, f"compaction overflow: {cnt.max()} > {KCAP}"

    wcat = np.concatenate([np.asarray(inputs["Wk"]), np.asarray(inputs["Wb"]),
                           np.asarray(inputs["Ws"]), np.asarray(inputs["Wg"])],
                          axis=0).T  # [C, 69]
    # swizzle [C, 69] -> [P, C//P, 69] with c = k*128+p so the DMA reads one
    # contiguous 2.2KB run per partition
    wcat = np.ascontiguousarray(
        wcat.reshape(C // P, P, 69).transpose(1, 0, 2), dtype=np.float32)
    bcat = np.concatenate([np.asarray(inputs["bk"]), np.asarray(inputs["bb"]),
                           np.asarray(inputs["bs"]),
                           np.asarray(inputs["bg"])]).astype(np.float32)
    bcat_rep = np.ascontiguousarray(np.broadcast_to(bcat, (BL, 69)))

    # v[m] = w[(m-1024) % N]; extended with KT-1 wrap elements
    v = np.concatenate([prw[:, N // 2:], prw[:, :N // 2]], axis=1)
    wext = np.ascontiguousarray(
        np.concatenate([v, v[:, :KT - 1]], axis=1).astype(np.float16))

    ident = np.eye(P, dtype=np.float32)
    ksqn = np.ascontiguousarray(np.broadcast_to(
        -(np.arange(KT, dtype=np.float32) ** 2), (BL, KT)), dtype=np.float32)
    # slot order: gather writes slot i at (partition i%16, free i//16) of the
    # [16, KCAP/16] tile; the DRAM bounce stores linear j = p*16+f and the
    # cm2 reload maps j = 2*p2+k. slotid = gather index of each (p2, k).
    j = (np.arange(P, dtype=np.int64)[:, None] * KCH
         + np.arange(KCH, dtype=np.int64)[None, :])
    slotid = np.ascontiguousarray(
        ((j % 16) * (KCAP // 16) + j // 16).astype(np.float32))

    in_maps = []
    for cidx in range(NCORES):
        rows = slice(cidx * BL, (cidx + 1) * BL)
        in_maps.append({
            "mem": np.ascontiguousarray(memory[rows].astype(np.float16)),
            "coT": np.ascontiguousarray(
                co[rows].T.reshape(C // P, P, BL).transpose(1, 0, 2)),
            "wcat": wcat,
            "bcat": bcat_rep,
            "wext": np.ascontiguousarray(wext[rows]),
            "u": np.ascontiguousarray(usage[rows]),
            "ksqn": ksqn,
            "ident": ident,
            "slotid": slotid,
        })
    return in_maps


def kernel(**inputs):
    return _run(inputs, trace=False)[0]


def _run(inputs, trace=False):
    from concourse.bass_utils import run_bass_kernel_spmd

    if "nc" not in _CACHE:
        _CACHE["nc"] = _build()
    nc = _CACHE["nc"]

    in_maps = _host_prep(inputs)
    res = run_bass_kernel_spmd(nc, in_maps, core_ids=list(range(NCORES)),
                               trace=trace)

    ww = np.concatenate([res.results[i]["o_ww"] for i in range(NCORES)], axis=0)
    cw = np.concatenate([res.results[i]["o_cw"] for i in range(NCORES)], axis=0)
    dw = np.concatenate([res.results[i]["o_dw"] for i in range(NCORES)], axis=0)
    al = np.concatenate([res.results[i]["o_al"] for i in range(NCORES)], axis=0)
    out = (ww.astype(np.float32), cw.astype(np.float32),
           dw.astype(np.float32), al.astype(np.float32))
    return out, res
